# revision 26
# baseline (speedup 1.0000x reference)
"""Trainium2 Bass kernel: diagonal complex linear recurrence (SSM scan).

out[t, d] = z_d * out[t-1, d] + x[t, d],  z_d = exp(-exp(size_d) + i*theta_d)
x: [T=8192, D=2048] f32 -> out complex64.

v4 strategy:
  - global twist frame (tables cover t=0..T-1 directly; no chunk-boundary
    carry rotations).
  - time de-interleaved mod 4 host-side: per strip of 4096 steps, columns
    are laid out [stream s=0..3][k=0..1023] with t = 4096p + 4k + s.
  - mod-4 decimated scan: PE builds G[k] = r^3 u0 + r^2 u1 + r u2 + u3 via
    diag-weight matmuls accumulating in PSUM; DVE scans only T/4 elements
    (decay r^4); streams 0..2 reconstructed with tensor_scalar_mul (4x/2x
    mode) + tensor_add (2x mode) pairs.
  - sin table eliminated: out_im = C.wim + S.wre is formed on PE as
    I x (C.wim) + (-I) x (NS.wre) with a negated identity weight.
  - untwist products alias the dead u/V tiles so the mid pool double-buffers
    across (strip, group) iterations without extra SBUF.
  - coarse DMAs (one per stream tile; first strip split for lead-in) to
    keep sync-engine issue time low.
"""

import os
import sys

import numpy as np

for _p in ("/opt/trn_rl_repo", "/root/.axon_site/_ro/trn_rl_repo"):
    if os.path.isdir(_p) and _p not in sys.path:
        sys.path.append(_p)

import concourse.bacc as bacc
import concourse.mybir as mybir
from concourse import bass_utils
from concourse.tile import TileContext

T = 8192
D = 2048
NCORES = 8
DS = D // NCORES          # 256 channels per core
G = DS // 128             # partition groups per core (2)
S = 4                     # de-interleave streams (decimation factor)
P = 2                     # strips
KS = T // (P * S)         # 1024 scan elements per stream per strip
SW = S * KS               # 4096 strip width
F32 = mybir.dt.float32
F16 = mybir.dt.float16

_PROGRAM = None


def _build_program():
    nc = bacc.Bacc("TRN2", target_bir_lowering=False)

    xT = nc.dram_tensor("xT", (DS, T), F16, kind="ExternalInput")
    ctab = nc.dram_tensor("ctab", (DS, T), F16, kind="ExternalInput")
    nstab = nc.dram_tensor("nstab", (DS, T), F16, kind="ExternalInput")
    rb4 = nc.dram_tensor("rb4", (DS, KS), F32, kind="ExternalInput")
    rsc = nc.dram_tensor("rsc", (DS, 1), F32, kind="ExternalInput")
    # weight wall: [g0r3, g0r2, g0r1, g1r3, g1r2, g1r1, I, -I] blocks
    wall = nc.dram_tensor("wall", (128, 8 * 128), F16, kind="ExternalInput")
    out_re = nc.dram_tensor("out_re", (DS, T), F16, kind="ExternalOutput")
    out_im = nc.dram_tensor("out_im", (DS, T), F16, kind="ExternalOutput")

    mult = mybir.AluOpType.mult
    add = mybir.AluOpType.add

    with TileContext(nc) as tc:
        with tc.tile_pool(name="tabs", bufs=1) as tpool, \
             tc.tile_pool(name="stream", bufs=3) as spool, \
             tc.tile_pool(name="mid", bufs=2) as mpool, \
             tc.tile_pool(name="w3p", bufs=1) as wpool, \
             tc.tile_pool(name="ovp", bufs=4) as opool, \
             tc.tile_pool(name="gps", bufs=1, space="PSUM") as gpool, \
             tc.tile_pool(name="aps", bufs=1, space="PSUM") as apool:

            # first-iteration stream tiles loaded in quarters, issued before
            # the resident loads, so the DVE starts as early as possible.
            first = {}
            for nm in ("xt", "ct", "nst"):
                first[nm] = spool.tile([128, SW], F16, name=nm, tag=nm)
            for hh in range(4):
                sl = slice(hh * SW // 4, (hh + 1) * SW // 4)
                for nm, src in (("xt", xT), ("ct", ctab), ("nst", nstab)):
                    nc.sync.dma_start(first[nm][:, sl], src[0:128, sl])

            # resident: decay tables, weight wall
            wallt = tpool.tile([128, 8 * 128], F16, name="wallt")
            nc.sync.dma_start(wallt[:], wall[:])

            def wmat(i):
                return wallt[:, i * 128:(i + 1) * 128]

            eye_w, neye_w = wmat(6), wmat(7)

            rb4t, rsct = [], []
            for g in range(G):
                pg = slice(g * 128, (g + 1) * 128)
                rb = tpool.tile([128, KS], F32, name=f"rb4t{g}")
                rs = tpool.tile([128, 1], F32, name=f"rsct{g}")
                nc.sync.dma_start(rb[:], rb4[pg, :])
                nc.sync.dma_start(rs[:], rsc[pg, :])
                rb4t.append(rb)
                rsct.append(rs)

            # scan-chain tiles: [pad, cc | w3 strip0 (1024) | w3 strip1 (1024)]
            W3 = {}
            for g in range(G):
                for c in range(2):
                    w = wpool.tile([128, 2 + P * KS], F16, name=f"W3_{g}{c}")
                    nc.vector.memset(w[:, 1:2], 0.0)
                    W3[(g, c)] = w

            for p in range(P):
                for g in range(G):
                    pg = slice(g * 128, (g + 1) * 128)
                    base = p * SW

                    if p == 0 and g == 0:
                        xt, ct, nst = first["xt"], first["ct"], first["nst"]
                    else:
                        xt = spool.tile([128, SW], F16, name="xt", tag="xt")
                        ct = spool.tile([128, SW], F16, name="ct", tag="ct")
                        nst = spool.tile([128, SW], F16, name="nst", tag="nst")
                        nc.sync.dma_start(xt[:], xT[pg, base:base + SW])
                        nc.sync.dma_start(ct[:], ctab[pg, base:base + SW])
                        nc.sync.dma_start(nst[:], nstab[pg, base:base + SW])
                    ure = mpool.tile([128, SW], F16, name="ure", tag="ure")
                    uim = mpool.tile([128, SW], F16, name="uim", tag="uim")

                    # twist, sliced for pipelining (finer on first iteration)
                    nh = 4 if (p == 0 and g == 0) else 2
                    for hh in range(nh):
                        sl = slice(hh * SW // nh, (hh + 1) * SW // nh)
                        nc.vector.tensor_mul(ure[:, sl], xt[:, sl], ct[:, sl])
                        nc.vector.tensor_mul(uim[:, sl], xt[:, sl], nst[:, sl])

                    V = [None, None]
                    for c, u in ((0, ure), (1, uim)):
                        w3t = W3[(g, c)]
                        # G[k] = r^3 u0 + r^2 u1 + r u2 + u3  (PE, PSUM acc),
                        # in two half-width PSUM tiles so each half frees for
                        # the next iteration as soon as its scan half is done.
                        so = 2 + p * KS
                        for hi, h in enumerate((0, 512)):
                            gp = gpool.tile([128, 512], F32, name=f"G{hi}{c}",
                                            tag=f"G{hi}{c}")
                            nc.tensor.matmul(gp[:], wmat(3 * g),
                                             u[:, 0 * KS + h:0 * KS + h + 512],
                                             start=True, stop=False)
                            nc.tensor.matmul(gp[:], wmat(3 * g + 1),
                                             u[:, 1 * KS + h:1 * KS + h + 512],
                                             start=False, stop=False)
                            nc.tensor.matmul(gp[:], wmat(3 * g + 2),
                                             u[:, 2 * KS + h:2 * KS + h + 512],
                                             start=False, stop=False)
                            nc.tensor.matmul(gp[:], eye_w,
                                             u[:, 3 * KS + h:3 * KS + h + 512],
                                             start=False, stop=True)
                            # decimated scan (decay r^4) of stream 3, chained
                            # across the two halves
                            if h == 0:
                                init = 0.0 if p == 0 else w3t[:, so - 1:so]
                            else:
                                init = w3t[:, so + h - 1:so + h]
                            nc.vector.tensor_tensor_scan(
                                w3t[:, so + h:so + h + 512], rb4t[g][:, 0:512],
                                gp[:], init, op0=mult, op1=add)

                        # reconstruct streams 0..2 into V (stream-major);
                        # scale-by-r runs on the scalar engine (per-partition
                        # scale), the adds on DVE.
                        ident = mybir.ActivationFunctionType.Identity
                        Vc = mpool.tile([128, SW], F16, name=f"V{c}", tag=f"V{c}")
                        wds = w3t[:, so - 1:so - 1 + KS]
                        tmp = mpool.tile([128, KS], F16, name=f"tm{c}", tag=f"tm{c}")
                        nc.scalar.activation(tmp[:], wds, ident, scale=rsct[g][:])
                        nc.vector.tensor_add(Vc[:, 0:KS], tmp[:], u[:, 0:KS])
                        tmp2 = mpool.tile([128, KS], F16, name=f"tn{c}", tag=f"tn{c}")
                        nc.scalar.activation(tmp2[:], Vc[:, 0:KS], ident,
                                             scale=rsct[g][:])
                        nc.vector.tensor_add(Vc[:, KS:2 * KS], tmp2[:], u[:, KS:2 * KS])
                        tmp3 = mpool.tile([128, KS], F16, name=f"to{c}", tag=f"to{c}")
                        nc.scalar.activation(tmp3[:], Vc[:, KS:2 * KS], ident,
                                             scale=rsct[g][:])
                        nc.vector.tensor_add(Vc[:, 2 * KS:3 * KS], tmp3[:],
                                             u[:, 2 * KS:3 * KS])
                        V[c] = Vc

                    Vre, Vim = V
                    # untwist products, aliased onto dead u/V tiles:
                    #   t1 = C.wre -> ure   t4 = C.wim -> uim
                    #   t3 = NS.wre -> Vre  t2 = NS.wim -> Vim
                    # streams 0..2 read V, stream 3 reads the scan output in
                    # its W3 region directly (saves a scalar copy).
                    for (a, b) in ((0, 2 * KS), (2 * KS, 3 * KS)):
                        sl = slice(a, b)
                        nc.vector.tensor_mul(ure[:, sl], ct[:, sl], Vre[:, sl])
                        nc.vector.tensor_mul(uim[:, sl], ct[:, sl], Vim[:, sl])
                        nc.vector.tensor_mul(Vre[:, sl], nst[:, sl], Vre[:, sl])
                        nc.vector.tensor_mul(Vim[:, sl], nst[:, sl], Vim[:, sl])
                    s3 = slice(3 * KS, 4 * KS)
                    so = 2 + p * KS
                    w3re_ap = W3[(g, 0)][:, so:so + KS]
                    w3im_ap = W3[(g, 1)][:, so:so + KS]
                    nc.vector.tensor_mul(ure[:, s3], ct[:, s3], w3re_ap)
                    nc.vector.tensor_mul(uim[:, s3], ct[:, s3], w3im_ap)
                    nc.vector.tensor_mul(Vre[:, s3], nst[:, s3], w3re_ap)
                    nc.vector.tensor_mul(Vim[:, s3], nst[:, s3], w3im_ap)
                    t1, t4, t3, t2 = ure, uim, Vre, Vim

                    # out_re = t1 + t2 ; out_im = t4 - t3   (PE adds)
                    for h in range(0, SW, 1024):
                        psA = apool.tile([128, 1024], F32, name="psA", tag="psA")
                        psB = apool.tile([128, 1024], F32, name="psB", tag="psB")
                        for qq in range(0, 1024, 512):
                            hs = slice(h + qq, h + qq + 512)
                            qs = slice(qq, qq + 512)
                            nc.tensor.matmul(psA[:, qs], eye_w, t1[:, hs],
                                             start=True, stop=False)
                            nc.tensor.matmul(psA[:, qs], eye_w, t2[:, hs],
                                             start=False, stop=True)
                            nc.tensor.matmul(psB[:, qs], eye_w, t4[:, hs],
                                             start=True, stop=False)
                            nc.tensor.matmul(psB[:, qs], neye_w, t3[:, hs],
                                             start=False, stop=True)
                        ovA = opool.tile([128, 1024], F16, name="ovA", tag="ovA")
                        nc.scalar.copy(ovA[:], psA[:])
                        nc.sync.dma_start(out_re[pg, base + h:base + h + 1024], ovA[:])
                        ovB = opool.tile([128, 1024], F16, name="ovB", tag="ovB")
                        nc.scalar.copy(ovB[:], psB[:])
                        nc.sync.dma_start(out_im[pg, base + h:base + h + 1024], ovB[:])

    nc.compile()
    return nc


def _get_program():
    global _PROGRAM
    if _PROGRAM is None:
        _PROGRAM = _build_program()
    return _PROGRAM


def _dev_perm():
    # device column j = p*SW + s*KS + k  <->  t = p*SW + S*k + s
    t_of_dev = np.empty(T, np.int64)
    for p in range(P):
        for s in range(S):
            k = np.arange(KS)
            t_of_dev[p * SW + s * KS + k] = p * SW + S * k + s
    return t_of_dev


def _host_prep(x, size, theta):
    size64 = np.asarray(size, np.float64)
    theta64 = np.asarray(theta, np.float64)
    r64 = np.exp(-np.exp(size64))

    tdev = _dev_perm()
    ang = theta64[:, None] * tdev[None, :].astype(np.float64)
    ctab = np.cos(ang).astype(np.float16)
    nstab = (-np.sin(ang)).astype(np.float16)

    rb4 = np.broadcast_to((r64 ** 4)[:, None].astype(np.float32),
                          (D, KS)).copy()
    rsc = r64[:, None].astype(np.float32)

    xh = np.asarray(x, np.float32).astype(np.float16)
    eye = np.eye(128, dtype=np.float16)

    in_maps = []
    for cidx in range(NCORES):
        sl = slice(cidx * DS, (cidx + 1) * DS)
        xTc = np.ascontiguousarray(xh[:, sl].T)[:, tdev]
        r3 = (r64[sl] ** 3).astype(np.float16)
        r2 = (r64[sl] ** 2).astype(np.float16)
        r1 = r64[sl].astype(np.float16)
        wall = np.zeros((128, 8 * 128), np.float16)
        for g in range(G):
            gs = slice(g * 128, (g + 1) * 128)
            wall[:, (3 * g + 0) * 128:(3 * g + 1) * 128] = np.diag(r3[gs])
            wall[:, (3 * g + 1) * 128:(3 * g + 2) * 128] = np.diag(r2[gs])
            wall[:, (3 * g + 2) * 128:(3 * g + 3) * 128] = np.diag(r1[gs])
        wall[:, 6 * 128:7 * 128] = eye
        wall[:, 7 * 128:8 * 128] = -eye
        in_maps.append({
            "xT": np.ascontiguousarray(xTc),
            "ctab": np.ascontiguousarray(ctab[sl]),
            "nstab": np.ascontiguousarray(nstab[sl]),
            "rb4": np.ascontiguousarray(rb4[sl]),
            "rsc": np.ascontiguousarray(rsc[sl]),
            "wall": wall,
        })
    return in_maps


def _assemble(results):
    tdev = _dev_perm()
    out = np.empty((T, D), np.complex64)
    plane = np.empty((DS, T), np.float32)
    for cidx, res in enumerate(results):
        sl = slice(cidx * DS, (cidx + 1) * DS)
        cplx = np.empty((DS, T), np.complex64)
        plane[:, tdev] = res["out_re"].astype(np.float32)
        cplx.real = plane
        plane[:, tdev] = res["out_im"].astype(np.float32)
        cplx.imag = plane
        out[:, sl] = cplx.T
    return out


def run(x, size, theta, trace=False, **spmd_kwargs):
    nc = _get_program()
    in_maps = _host_prep(x, size, theta)
    res = bass_utils.run_bass_kernel_spmd(
        nc, in_maps, core_ids=list(range(NCORES)), trace=trace, **spmd_kwargs)
    return _assemble(res.results), res


def kernel(x, size, theta):
    out, _ = run(x, size, theta, trace=False)
    return out


# revision 30
# speedup vs baseline: 1.0330x; 1.0330x over previous
"""Trainium2 Bass kernel: diagonal complex linear recurrence (SSM scan).

out[t, d] = z_d * out[t-1, d] + x[t, d],  z_d = exp(-exp(size_d) + i*theta_d)
x: [T=8192, D=2048] f32 -> out complex64.

v4 strategy:
  - global twist frame (tables cover t=0..T-1 directly; no chunk-boundary
    carry rotations).
  - time de-interleaved mod 4 host-side: per strip of 4096 steps, columns
    are laid out [stream s=0..3][k=0..1023] with t = 4096p + 4k + s.
  - mod-4 decimated scan: PE builds G[k] = r^3 u0 + r^2 u1 + r u2 + u3 via
    diag-weight matmuls accumulating in PSUM; DVE scans only T/4 elements
    (decay r^4); streams 0..2 reconstructed with tensor_scalar_mul (4x/2x
    mode) + tensor_add (2x mode) pairs.
  - sin table eliminated: out_im = C.wim + S.wre is formed on PE as
    I x (C.wim) + (-I) x (NS.wre) with a negated identity weight.
  - untwist products alias the dead u/V tiles so the mid pool double-buffers
    across (strip, group) iterations without extra SBUF.
  - coarse DMAs (one per stream tile; first strip split for lead-in) to
    keep sync-engine issue time low.
"""

import os
import sys

import numpy as np

for _p in ("/opt/trn_rl_repo", "/root/.axon_site/_ro/trn_rl_repo"):
    if os.path.isdir(_p) and _p not in sys.path:
        sys.path.append(_p)

import concourse.bacc as bacc
import concourse.mybir as mybir
from concourse import bass_utils
from concourse.tile import TileContext

T = 8192
D = 2048
NCORES = 8
DS = D // NCORES          # 256 channels per core
G = DS // 128             # partition groups per core (2)
S = 4                     # de-interleave streams (decimation factor)
P = 2                     # strips
KS = T // (P * S)         # 1024 scan elements per stream per strip
SW = S * KS               # 4096 strip width
F32 = mybir.dt.float32
F16 = mybir.dt.float16

_PROGRAM = None


def _build_program():
    nc = bacc.Bacc("TRN2", target_bir_lowering=False)

    xT = nc.dram_tensor("xT", (DS, T), F16, kind="ExternalInput")
    ctab = nc.dram_tensor("ctab", (DS, T), F16, kind="ExternalInput")
    nstab = nc.dram_tensor("nstab", (DS, T), F16, kind="ExternalInput")
    rsc = nc.dram_tensor("rsc", (DS, 2), F32, kind="ExternalInput")
    # weight wall: [g0r3, g0r2, g0r1, g1r3, g1r2, g1r1, I, -I] blocks
    wall = nc.dram_tensor("wall", (128, 8 * 128), F16, kind="ExternalInput")
    out_re = nc.dram_tensor("out_re", (DS, T), F16, kind="ExternalOutput")
    out_im = nc.dram_tensor("out_im", (DS, T), F16, kind="ExternalOutput")

    mult = mybir.AluOpType.mult
    add = mybir.AluOpType.add

    with TileContext(nc) as tc:
        with tc.tile_pool(name="tabs", bufs=1) as tpool, \
             tc.tile_pool(name="stream", bufs=3) as spool, \
             tc.tile_pool(name="mid", bufs=2) as mpool, \
             tc.tile_pool(name="w3p", bufs=1) as wpool, \
             tc.tile_pool(name="ovp", bufs=4) as opool, \
             tc.tile_pool(name="gps", bufs=1, space="PSUM") as gpool, \
             tc.tile_pool(name="aps", bufs=1, space="PSUM") as apool:

            # first-iteration stream tiles loaded in quarters, issued before
            # the resident loads, so the DVE starts as early as possible.
            first = {}
            for nm in ("xt", "ct", "nst"):
                first[nm] = spool.tile([128, SW], F16, name=nm, tag=nm)
            for hh in range(4):
                sl = slice(hh * SW // 4, (hh + 1) * SW // 4)
                for nm, src in (("xt", xT), ("ct", ctab), ("nst", nstab)):
                    nc.sync.dma_start(first[nm][:, sl], src[0:128, sl])

            # resident: decay tables, weight wall
            wallt = tpool.tile([128, 8 * 128], F16, name="wallt")
            nc.sync.dma_start(wallt[:], wall[:])

            def wmat(i):
                return wallt[:, i * 128:(i + 1) * 128]

            eye_w, neye_w = wmat(6), wmat(7)

            # rb4 (r^4 broadcast along k) is built on-device: memset 1.0 then
            # in-place per-partition multiply by r^4 — saves 1 MiB of DMA
            # during the bandwidth-bound startup ramp.
            rb4t, rsct = [], []
            for g in range(G):
                pg = slice(g * 128, (g + 1) * 128)
                rs = tpool.tile([128, 2], F32, name=f"rsct{g}")
                nc.sync.dma_start(rs[:], rsc[pg, :])
                rb = tpool.tile([128, KS], F32, name=f"rb4t{g}")
                nc.vector.memset(rb[:], 1.0)
                nc.vector.tensor_scalar_mul(rb[:], rb[:], rs[:, 1:2])
                rb4t.append(rb)
                rsct.append(rs)

            # scan-chain tiles: [pad, cc | w3 strip0 (1024) | w3 strip1 (1024)]
            W3 = {}
            for g in range(G):
                for c in range(2):
                    w = wpool.tile([128, 2 + P * KS], F16, name=f"W3_{g}{c}")
                    nc.vector.memset(w[:, 1:2], 0.0)
                    W3[(g, c)] = w

            for p in range(P):
                for g in range(G):
                    pg = slice(g * 128, (g + 1) * 128)
                    base = p * SW

                    if p == 0 and g == 0:
                        xt, ct, nst = first["xt"], first["ct"], first["nst"]
                    else:
                        xt = spool.tile([128, SW], F16, name="xt", tag="xt")
                        ct = spool.tile([128, SW], F16, name="ct", tag="ct")
                        nst = spool.tile([128, SW], F16, name="nst", tag="nst")
                        nc.sync.dma_start(xt[:], xT[pg, base:base + SW])
                        nc.sync.dma_start(ct[:], ctab[pg, base:base + SW])
                        nc.sync.dma_start(nst[:], nstab[pg, base:base + SW])
                    ure = mpool.tile([128, SW], F16, name="ure", tag="ure")
                    uim = mpool.tile([128, SW], F16, name="uim", tag="uim")

                    # twist, sliced for pipelining (finer on first iteration)
                    nh = 4 if (p == 0 and g == 0) else 2
                    for hh in range(nh):
                        sl = slice(hh * SW // nh, (hh + 1) * SW // nh)
                        nc.vector.tensor_mul(ure[:, sl], xt[:, sl], ct[:, sl])
                        nc.vector.tensor_mul(uim[:, sl], xt[:, sl], nst[:, sl])

                    V = [None, None]
                    for c, u in ((0, ure), (1, uim)):
                        w3t = W3[(g, c)]
                        # G[k] = r^3 u0 + r^2 u1 + r u2 + u3  (PE, PSUM acc)
                        gp = gpool.tile([128, KS], F32, name=f"G{c}", tag=f"G{c}")
                        for h in range(0, KS, 512):
                            hs = slice(h, h + 512)
                            nc.tensor.matmul(gp[:, hs], wmat(3 * g),
                                             u[:, 0 * KS + h:0 * KS + h + 512],
                                             start=True, stop=False)
                            nc.tensor.matmul(gp[:, hs], wmat(3 * g + 1),
                                             u[:, 1 * KS + h:1 * KS + h + 512],
                                             start=False, stop=False)
                            nc.tensor.matmul(gp[:, hs], wmat(3 * g + 2),
                                             u[:, 2 * KS + h:2 * KS + h + 512],
                                             start=False, stop=False)
                            nc.tensor.matmul(gp[:, hs], eye_w,
                                             u[:, 3 * KS + h:3 * KS + h + 512],
                                             start=False, stop=True)

                        # decimated scan (decay r^4) of stream 3
                        so = 2 + p * KS
                        init = 0.0 if p == 0 else w3t[:, so - 1:so]
                        nc.vector.tensor_tensor_scan(
                            w3t[:, so:so + KS], rb4t[g][:], gp[:], init,
                            op0=mult, op1=add)

                        # reconstruct streams 0..2 into V (stream-major);
                        # scale-by-r runs on the scalar engine (per-partition
                        # scale), the adds on DVE.
                        ident = mybir.ActivationFunctionType.Identity
                        Vc = mpool.tile([128, SW], F16, name=f"V{c}", tag=f"V{c}")
                        wds = w3t[:, so - 1:so - 1 + KS]
                        tmp = mpool.tile([128, KS], F16, name=f"tm{c}", tag=f"tm{c}")
                        nc.scalar.activation(tmp[:], wds, ident, scale=rsct[g][:, 0:1])
                        nc.vector.tensor_add(Vc[:, 0:KS], tmp[:], u[:, 0:KS])
                        tmp2 = mpool.tile([128, KS], F16, name=f"tn{c}", tag=f"tn{c}")
                        nc.scalar.activation(tmp2[:], Vc[:, 0:KS], ident,
                                             scale=rsct[g][:, 0:1])
                        nc.vector.tensor_add(Vc[:, KS:2 * KS], tmp2[:], u[:, KS:2 * KS])
                        tmp3 = mpool.tile([128, KS], F16, name=f"to{c}", tag=f"to{c}")
                        nc.scalar.activation(tmp3[:], Vc[:, KS:2 * KS], ident,
                                             scale=rsct[g][:, 0:1])
                        nc.vector.tensor_add(Vc[:, 2 * KS:3 * KS], tmp3[:],
                                             u[:, 2 * KS:3 * KS])
                        V[c] = Vc

                    Vre, Vim = V
                    # untwist products, aliased onto dead u/V tiles:
                    #   t1 = C.wre -> ure   t4 = C.wim -> uim
                    #   t3 = NS.wre -> Vre  t2 = NS.wim -> Vim
                    # streams 0..2 read V, stream 3 reads the scan output in
                    # its W3 region directly (saves a scalar copy).
                    for (a, b) in ((0, 2 * KS), (2 * KS, 3 * KS)):
                        sl = slice(a, b)
                        nc.vector.tensor_mul(ure[:, sl], ct[:, sl], Vre[:, sl])
                        nc.vector.tensor_mul(uim[:, sl], ct[:, sl], Vim[:, sl])
                        nc.vector.tensor_mul(Vre[:, sl], nst[:, sl], Vre[:, sl])
                        nc.vector.tensor_mul(Vim[:, sl], nst[:, sl], Vim[:, sl])
                    s3 = slice(3 * KS, 4 * KS)
                    so = 2 + p * KS
                    w3re_ap = W3[(g, 0)][:, so:so + KS]
                    w3im_ap = W3[(g, 1)][:, so:so + KS]
                    nc.vector.tensor_mul(ure[:, s3], ct[:, s3], w3re_ap)
                    nc.vector.tensor_mul(uim[:, s3], ct[:, s3], w3im_ap)
                    nc.vector.tensor_mul(Vre[:, s3], nst[:, s3], w3re_ap)
                    nc.vector.tensor_mul(Vim[:, s3], nst[:, s3], w3im_ap)
                    t1, t4, t3, t2 = ure, uim, Vre, Vim

                    # out_re = t1 + t2 ; out_im = t4 - t3   (PE adds)
                    for h in range(0, SW, 1024):
                        psA = apool.tile([128, 1024], F32, name="psA", tag="psA")
                        psB = apool.tile([128, 1024], F32, name="psB", tag="psB")
                        for qq in range(0, 1024, 512):
                            hs = slice(h + qq, h + qq + 512)
                            qs = slice(qq, qq + 512)
                            nc.tensor.matmul(psA[:, qs], eye_w, t1[:, hs],
                                             start=True, stop=False)
                            nc.tensor.matmul(psA[:, qs], eye_w, t2[:, hs],
                                             start=False, stop=True)
                            nc.tensor.matmul(psB[:, qs], eye_w, t4[:, hs],
                                             start=True, stop=False)
                            nc.tensor.matmul(psB[:, qs], neye_w, t3[:, hs],
                                             start=False, stop=True)
                        ovA = opool.tile([128, 1024], F16, name="ovA", tag="ovA")
                        nc.scalar.copy(ovA[:], psA[:])
                        nc.sync.dma_start(out_re[pg, base + h:base + h + 1024], ovA[:])
                        ovB = opool.tile([128, 1024], F16, name="ovB", tag="ovB")
                        nc.scalar.copy(ovB[:], psB[:])
                        nc.sync.dma_start(out_im[pg, base + h:base + h + 1024], ovB[:])

    nc.compile()
    return nc


def _get_program():
    global _PROGRAM
    if _PROGRAM is None:
        _PROGRAM = _build_program()
    return _PROGRAM


def _dev_perm():
    # device column j = p*SW + s*KS + k  <->  t = p*SW + S*k + s
    t_of_dev = np.empty(T, np.int64)
    for p in range(P):
        for s in range(S):
            k = np.arange(KS)
            t_of_dev[p * SW + s * KS + k] = p * SW + S * k + s
    return t_of_dev


def _host_prep(x, size, theta):
    size64 = np.asarray(size, np.float64)
    theta64 = np.asarray(theta, np.float64)
    r64 = np.exp(-np.exp(size64))

    tdev = _dev_perm()
    ang = theta64[:, None] * tdev[None, :].astype(np.float64)
    ctab = np.cos(ang).astype(np.float16)
    nstab = (-np.sin(ang)).astype(np.float16)

    rsc = np.stack([r64, r64 ** 4], axis=1).astype(np.float32)

    xh = np.asarray(x, np.float32).astype(np.float16)
    eye = np.eye(128, dtype=np.float16)

    in_maps = []
    for cidx in range(NCORES):
        sl = slice(cidx * DS, (cidx + 1) * DS)
        xTc = np.ascontiguousarray(xh[:, sl].T)[:, tdev]
        r3 = (r64[sl] ** 3).astype(np.float16)
        r2 = (r64[sl] ** 2).astype(np.float16)
        r1 = r64[sl].astype(np.float16)
        wall = np.zeros((128, 8 * 128), np.float16)
        for g in range(G):
            gs = slice(g * 128, (g + 1) * 128)
            wall[:, (3 * g + 0) * 128:(3 * g + 1) * 128] = np.diag(r3[gs])
            wall[:, (3 * g + 1) * 128:(3 * g + 2) * 128] = np.diag(r2[gs])
            wall[:, (3 * g + 2) * 128:(3 * g + 3) * 128] = np.diag(r1[gs])
        wall[:, 6 * 128:7 * 128] = eye
        wall[:, 7 * 128:8 * 128] = -eye
        in_maps.append({
            "xT": np.ascontiguousarray(xTc),
            "ctab": np.ascontiguousarray(ctab[sl]),
            "nstab": np.ascontiguousarray(nstab[sl]),
            "rsc": np.ascontiguousarray(rsc[sl]),
            "wall": wall,
        })
    return in_maps


def _assemble(results):
    tdev = _dev_perm()
    out = np.empty((T, D), np.complex64)
    plane = np.empty((DS, T), np.float32)
    for cidx, res in enumerate(results):
        sl = slice(cidx * DS, (cidx + 1) * DS)
        cplx = np.empty((DS, T), np.complex64)
        plane[:, tdev] = res["out_re"].astype(np.float32)
        cplx.real = plane
        plane[:, tdev] = res["out_im"].astype(np.float32)
        cplx.imag = plane
        out[:, sl] = cplx.T
    return out


def run(x, size, theta, trace=False, **spmd_kwargs):
    nc = _get_program()
    in_maps = _host_prep(x, size, theta)
    res = bass_utils.run_bass_kernel_spmd(
        nc, in_maps, core_ids=list(range(NCORES)), trace=trace, **spmd_kwargs)
    return _assemble(res.results), res


def kernel(x, size, theta):
    out, _ = run(x, size, theta, trace=False)
    return out
